# revision 5
# baseline (speedup 1.0000x reference)
"""Trainium2 Bass kernel for nn_Conv2d_layer_36584531427330.

Computes: conv_transpose2d(x, w, stride=2) -> depthwise 4x4 FIR ([1,3,3,1]/8
separable, gain 4) -> +bias -> leaky_relu(0.2) * sqrt(2).
  x: (32, 512, 32, 32) f32 -> out: (32, 256, 64, 64) f32

Strategy (data-parallel over batch, 4 images per core on 8 cores):
- Stride-2 transposed conv decomposed into 4 output-parity phases
  (EE/EO/OE/OO) with 4/2/2/1 taps; each tap is a [K=128ic x M=128oc x N]
  matmul accumulated in PSUM (fp16 operands, fp32 accumulate). Matmul rhs
  uses 2-D free APs [rows x ncols] so no padded columns are computed.
- Phase outputs are evicted (scalar engine, conv gain + FIR 1/16 + sqrt(2)
  folded into the scale) into cg[128, 67, 67]: ROWS interleaved to match
  the upsampled grid (row 0/66 zero pads, E-phase rows odd, O-phase rows
  even) but COLUMNS parity-blocked (E cols 0..32, col 33 = pad, O cols
  34..65, col 66 = pad) so every evict writes contiguous 33-col runs —
  scattered SBUF writes measurably slow all engines sharing the ports.
- The separable FIR uses [1,3,3,1] = [1,1]*[1,1]*[1,1]. The H dimension
  (parity-blocked cols) takes 6 paired 2-tap adds; the V dimension
  (interleaved rows) is 3 plain shift-adds. All on the vector engine;
  gpsimd does nothing in steady state (its tensor ops contend with the
  DVE for SBUF ports: measured 4x DVE slowdown when overlapped).
- Bias + leaky relu via the exact relu trick (works for any bias):
    rt = relu(4u + 4bt); lk = u + rt; out = 0.2*lk + 0.2*bt  (bt=sqrt2*b)
  The final activation also x-deinterleaves into natural output order.
- Software pipelining: stage 2 (FIR+act+store) of slab s-1 is emitted after
  stage 1 (matmuls+evicts) of slab s. The FIR is row-bandable; the first
  and last slabs run banded (3 bands aligned to PSUM row-chunks) so the
  vector engine starts ~15us earlier and the drain tail is ~2 bands
  shorter.
"""

import numpy as np

import concourse.bass as bass
from concourse import bacc
import concourse.mybir as mybir
import concourse.tile as tile
from concourse.bass_utils import run_bass_kernel_spmd

N_CORES = 8
B, IC, OC, K = 32, 512, 256, 3
BPC = B // N_CORES          # images per core
ICC = IC // 128             # ic chunks
SQRT2 = 1.4142135623730951
PLANE = 34 * 34 + 34  # padded plane + overrun tail for row-chunk slices
GAIN = 1.0 / np.sqrt(IC * K * K)
S_EVICT = float(GAIN * SQRT2 / 16.0)

F16 = mybir.dt.float16
F32 = mybir.dt.float32

# phase: (name, ncols, taps[(dy,dx)], row_parity ai, col_block co, chunks)
# chunks are (r0, rn) with rn*ncols <= 512 (PSUM bank limit); chunk
# boundaries at 15/30 align with the FIR row bands.
CHUNKS_E = [(0, 15), (15, 15), (30, 3)]
CHUNKS_O = [(0, 15), (15, 15), (30, 2)]
PHASES = [
    ("EE", 33, [(0, 0), (0, 2), (2, 0), (2, 2)], 0, 0, CHUNKS_E),
    ("EO", 32, [(0, 1), (2, 1)], 0, 34, CHUNKS_E),
    ("OE", 33, [(1, 0), (1, 2)], 1, 0, CHUNKS_O),
    ("OO", 32, [(1, 1)], 1, 34, CHUNKS_O),
]
# FIR row bands (out-row ranges) aligned to the chunk boundaries
BANDS = [(0, 28), (28, 58), (58, 64)]


def _w_off(icc, dy, dx, half):
    return (((half * ICC + icc) * 3 + dy) * 3 + dx) * 128


def _build_nc():
    nc = bacc.Bacc(None, target_bir_lowering=False)
    xp = nc.dram_tensor("xp", [BPC, IC, PLANE], F16, kind="ExternalInput")
    wt = nc.dram_tensor("wt", [128, ICC * 3 * 3 * 2 * 128], F16, kind="ExternalInput")
    bias2 = nc.dram_tensor("bias2", [128, 4], F32, kind="ExternalInput")
    out = nc.dram_tensor("out", [BPC, OC, 64, 64], F32, kind="ExternalOutput")

    add = mybir.AluOpType.add

    with tile.TileContext(nc) as tc:
        with (
            tc.tile_pool(name="const", bufs=1) as cpool,
            tc.tile_pool(name="pers", bufs=1) as ppool,
            tc.tile_pool(name="xin", bufs=2) as xpool,
            tc.tile_pool(name="z", bufs=1) as zpool,
            tc.tile_pool(name="outp", bufs=2) as opool,
            tc.tile_pool(name="psum", bufs=8, space="PSUM") as pspool,
        ):
            w_sb = cpool.tile([128, ICC * 3 * 3 * 2 * 128], F16, name="w_sb")
            bias_sb = cpool.tile([128, 4], F32, name="bias_sb")
            # weights are half-major: 4 quarter loads in consumption order
            WQ = ICC * 3 * 3 * 2 * 128 // 4
            for q in range(4):
                nc.sync.dma_start(w_sb[:, q * WQ:(q + 1) * WQ],
                                  wt[:, q * WQ:(q + 1) * WQ])
            nc.scalar.dma_start(bias_sb[:], bias2[:])

            # persistent phase-grid tiles (manual double buffer; borders
            # zeroed once, only interiors are rewritten). Emitted after the
            # first x DMA triggers so they don't delay the critical load.
            cgs = [ppool.tile([128, 67, 67], F16, name=f"cg{i}") for i in range(2)]

            def emit_memsets():
                for cg in cgs:
                    nc.gpsimd.memset(cg[:, 0:1, :], 0.0)
                    nc.gpsimd.memset(cg[:, 66:67, :], 0.0)
                    nc.gpsimd.memset(cg[:, :, 33:34], 0.0)
                    nc.gpsimd.memset(cg[:, :, 66:67], 0.0)

            wz = cpool.tile([128, 512], F16, name="wz")
            nc.vector.memset(wz[:], 0.0)
            zz = zpool.tile([128, 67, 64], F16, name="za", tag="za")
            nc.vector.memset(zz[:, 0:1], 0.0)
            nc.vector.memset(zz[:, 66:67], 0.0)
            psw = pspool.tile([128, 512], F32, name="psw", tag="ps")
            for i in range(14):
                nc.tensor.matmul(psw[:], lhsT=wz[:, 0:128], rhs=wz[:],
                                 start=(i == 0), stop=(i == 13))

            x_sbs = [None, None]

            def load_x(img):
                x_sb = xpool.tile([128, ICC, PLANE], F16, name="x_sb",
                                  tag="x_sb")
                # trigger from otherwise-idle engine queues: dma_start
                # triggers cost ~0.7us serial issue time per engine queue,
                # so spreading them across queues parallelizes the head
                xv = xp[img].rearrange("(c p) f -> p c f", p=128)
                if img == 0:
                    # first image gates the whole pipeline: 8 pieces on two
                    # queues so the transfer finishes ~5us sooner
                    HP = PLANE // 2
                    xvh = xv.rearrange("p c (h f) -> p c h f", h=2)
                    xbh = x_sb.rearrange("p c (h f) -> p c h f", h=2)
                    for c in range(ICC):
                        nc.gpsimd.dma_start(xbh[:, c, 0], xvh[:, c, 0])
                        nc.scalar.dma_start(xbh[:, c, 1], xvh[:, c, 1])
                else:
                    for c in range(ICC):
                        nc.gpsimd.dma_start(x_sb[:, c:c + 1], xv[:, c:c + 1])
                x_sbs[img % 2] = x_sb

            def stage1_group(slab, g):
                """Matmul chains + evicts for chunk index g of all phases."""
                img, half = divmod(slab, 2)
                cg = cgs[slab % 2]
                x_sb = x_sbs[img % 2]
                # rows 1..66 viewed as (row, parity): E rows odd, O rows even
                cgr = cg[:, 1:67, :].rearrange("p (r a) c -> p r a c", a=2)
                for nm, ncols, taps, ai, co, chunks in PHASES:
                    r0, rn = chunks[g]
                    ps = pspool.tile([128, 512], F32, name="ps", tag="ps")
                    nmm = len(taps) * ICC
                    kk = 0
                    for dy, dx in taps:
                        ey = -1 if dy == 2 else 0
                        ex = -1 if dx == 2 else 0
                        for icc in range(ICC):
                            wsl = w_sb[:, _w_off(icc, dy, dx, half):
                                       _w_off(icc, dy, dx, half) + 128]
                            st = (r0 + ey + 1) * 34 + (ex + 1)
                            rhs = x_sb[:, icc, st:st + rn * 34].rearrange(
                                "p (r c) -> p r c", c=34)[:, :, 0:ncols]
                            nc.tensor.matmul(
                                ps[:, :rn * ncols],
                                lhsT=wsl,
                                rhs=rhs,
                                start=(kk == 0),
                                stop=(kk == nmm - 1),
                            )
                            kk += 1
                    psv = ps[:, :rn * ncols].rearrange(
                        "p (r c) -> p r c", c=ncols)
                    nc.scalar.mul(
                        cgr[:, r0:r0 + rn, ai, co:co + ncols], psv, S_EVICT)

            def stage2_band(slab, tiles, oa, ob):
                """FIR + bias/lrelu + store for out rows [oa, ob)."""
                img, half = divmod(slab, 2)
                cg = cgs[slab % 2]
                uc, wc, za, ac, bc, out_pre, rt, lk, out_f32 = tiles
                ze = min(ob + 2, 66) + 1   # cg/za row window [oa, ze)
                ae = min(ob + 1, 65) + 1   # a row window [oa, ae)
                # pad rows 0/66 of za are constant zero (set once at init),
                # so the H passes only compute real rows
                w = slice(max(oa, 1), min(ze, 66))
                # FIR-H: [1,1]^3 on parity-blocked cols (E_c = cg col c,
                # O_c = cg col 34+c; cols 33/66 are the zero pads)
                nc.vector.tensor_tensor(
                    uc[:, w, 0, :], cg[:, w, 33:66], cg[:, w, 0:33], add)
                nc.vector.tensor_tensor(
                    uc[:, w, 1, :], cg[:, w, 0:33], cg[:, w, 34:67], add)
                nc.vector.tensor_tensor(
                    wc[:, w, 0:33], uc[:, w, 0, :], uc[:, w, 1, :], add)
                nc.vector.tensor_tensor(
                    wc[:, w, 33:65], uc[:, w, 1, 0:32], uc[:, w, 0, 1:33], add)
                nc.vector.tensor_tensor(
                    za[:, w, 0:32], wc[:, w, 0:32], wc[:, w, 33:65], add)
                nc.vector.tensor_tensor(
                    za[:, w, 32:64], wc[:, w, 33:65], wc[:, w, 1:33], add)
                # FIR-V: [1,1]^3 on interleaved rows (za row 0 = pad,
                # 1+2q = zE_q, 2+2q = zO_q, row 66 = pad)
                nc.vector.tensor_tensor(
                    ac[:, oa:ae], za[:, oa:ae], za[:, oa + 1:ae + 1], add)
                nc.vector.tensor_tensor(
                    bc[:, oa:ob + 1], ac[:, oa:ob + 1], ac[:, oa + 1:ob + 2], add)
                nc.vector.tensor_tensor(
                    out_pre[:, oa:ob], bc[:, oa:ob], bc[:, oa + 1:ob + 1], add)
                # bias + leaky relu (exact): lk = u + relu(4u+4bt)
                upf = out_pre.rearrange("p y c -> p (y c)")[:, oa * 64:ob * 64]
                nc.scalar.activation(
                    rt[:, oa * 64:ob * 64], upf,
                    mybir.ActivationFunctionType.Relu,
                    bias=bias_sb[:, half:half + 1], scale=4.0)
                nc.vector.tensor_tensor(
                    lk[:, oa * 64:ob * 64], upf, rt[:, oa * 64:ob * 64], add)
                # 0.2*lk + 0.2bt to fp32, deinterleaving cols (out col
                # 2r+t <- lk col t*32+r within each row)
                ofv = out_f32.rearrange("p (y r t) -> p y t r", y=64, r=32)
                lkv = lk.rearrange("p (y t r) -> p y t r", t=2, y=64)
                nc.scalar.activation(
                    ofv[:, oa:ob], lkv[:, oa:ob],
                    mybir.ActivationFunctionType.Identity,
                    bias=bias_sb[:, 2 + half:3 + half], scale=0.2)
                nc.sync.dma_start(
                    out[img, half * 128:(half + 1) * 128]
                    .rearrange("o h w -> o (h w)")[:, oa * 64:ob * 64],
                    out_f32[:, oa * 64:ob * 64],
                )

            def stage2_tiles():
                uc = zpool.tile([128, 67, 2, 33], F16, name="uc", tag="uc")
                wc = zpool.tile([128, 67, 65], F16, name="wc", tag="wc")
                za = zpool.tile([128, 67, 64], F16, name="za", tag="za")
                ac = zpool.tile([128, 66, 64], F16, name="ac", tag="ac")
                bc = zpool.tile([128, 65, 64], F16, name="bc", tag="bc")
                out_pre = opool.tile([128, 64, 64], F16, name="out_pre",
                                     tag="out_pre")
                rt = opool.tile([128, 64 * 64], F16, name="rt", tag="rt",
                                bufs=1)
                lk = opool.tile([128, 64 * 64], F16, name="lk", tag="lk")
                out_f32 = opool.tile([128, 64 * 64], F32, name="out_f32",
                                     tag="out_f32", bufs=1)
                return (uc, wc, za, ac, bc, out_pre, rt, lk, out_f32)

            NSLAB = 2 * BPC
            # slab 0: banded, interleaved with its own stage-1 chunk groups
            load_x(0)
            emit_memsets()
            t0 = stage2_tiles()
            for g in range(3):
                stage1_group(0, g)
                stage2_band(0, t0, *BANDS[g])
            # slabs 1..NSLAB-2: stage1(s) then monolithic stage2(s-1)
            for slab in range(1, NSLAB - 1):
                img, half = divmod(slab, 2)
                if half == 0:
                    load_x(img)
                for g in range(3):
                    stage1_group(slab, g)
                if slab > 1:
                    stage2_band(slab - 1, stage2_tiles(), 0, 64)
            # tail: the two remaining stage2s run banded, chasing the last
            # slab's chunk groups, so the post-matmul drain is ~2 bands
            t6 = stage2_tiles()
            t7 = stage2_tiles()
            stage1_group(NSLAB - 1, 0)
            stage2_band(NSLAB - 2, t6, *BANDS[0])
            stage1_group(NSLAB - 1, 1)
            stage2_band(NSLAB - 2, t6, *BANDS[1])
            stage2_band(NSLAB - 1, t7, *BANDS[0])
            stage1_group(NSLAB - 1, 2)
            stage2_band(NSLAB - 2, t6, *BANDS[2])
            stage2_band(NSLAB - 1, t7, *BANDS[1])
            stage2_band(NSLAB - 1, t7, *BANDS[2])
    nc.finalize()
    return nc


_NC_CACHE = None


def _get_nc():
    global _NC_CACHE
    if _NC_CACHE is None:
        _NC_CACHE = _build_nc()
    return _NC_CACHE


def _prep_inputs(x, weight, bias):
    x = np.asarray(x, dtype=np.float32)
    weight = np.asarray(weight, dtype=np.float32)
    bias = np.asarray(bias, dtype=np.float32)

    t = weight.reshape(2, 128, ICC, 128, 3, 3)       # (half, ocl, icc, icp, dy, dx)
    t = np.transpose(t, (3, 0, 2, 4, 5, 1))          # (icp, half, icc, dy, dx, ocl)
    wt_host = np.ascontiguousarray(t.reshape(128, -1)).astype(np.float16)

    bh = (bias * np.float32(SQRT2)).reshape(2, 128).T    # (128, half)
    bias2_host = np.ascontiguousarray(
        np.concatenate([4.0 * bh, 0.2 * bh], axis=1)
    ).astype(np.float32)

    x16 = x.astype(np.float16)
    in_maps = []
    for c in range(N_CORES):
        xp_host = np.zeros((BPC, IC, PLANE), np.float16)
        pl = np.zeros((BPC, IC, 34, 34), np.float16)
        pl[:, :, 1:33, 1:33] = x16[c * BPC:(c + 1) * BPC]
        xp_host[:, :, :34 * 34] = pl.reshape(BPC, IC, -1)
        in_maps.append({"xp": xp_host, "wt": wt_host, "bias2": bias2_host})
    return in_maps


def _execute(x, weight, bias, trace=False):
    nc = _get_nc()
    in_maps = _prep_inputs(x, weight, bias)
    res = run_bass_kernel_spmd(nc, in_maps, core_ids=list(range(N_CORES)),
                               trace=trace)
    out = np.concatenate([r["out"] for r in res.results], axis=0)
    return out, res


def kernel(x, weight, bias):
    out, _ = _execute(x, weight, bias, trace=False)
    return out
